# revision 12
# baseline (speedup 1.0000x reference)
"""Trainium2 Bass kernel for nn_CountingDecoder (MoE-routed dynamic-conv decoder).

Strategy (per core, data-parallel over batch: 16 samples / 8 cores = 2 each):
  1. Routing: softmax(mean(x) @ router_w.T + b) on-device (tiny fp32 matmuls).
  2. Dynamic conv (3x3 + 5x5 + 7x7, 128->512 ch) = for each of the 83 kernel
     offsets, a [K=128, M=128, N=512] matmul of the per-sample synthesized
     weight slice against a shifted window of the padded input, accumulated in
     fp32 PSUM.  Per-sample weights are synthesized on the vector engine with
     3 fused scalar_tensor_tensor ops per offset using the base+delta form
     c = W0 + sum_e r_e * (W_e - W0)   (softmax weights sum to 1).
  3. The 43.5MB bf16 expert-delta bank streams from HBM exactly once: the
     output channels are split into two passes so PSUM (8 banks) can hold both
     samples' accumulators for the half [2 samples x 256 ch x 1024 px].
  4. Channel attention: per-channel sums come free via activation accum_out
     during the PSUM->SBUF drain; the sigmoid gate is folded into the pred
     1x1-conv weights (scale lhsT rows) instead of scaling y [512,1024].
  5. Mask gate (1x1 conv to 1 ch) is applied after the pred matmul on
     [111,1024] instead of [512,1024]; sigmoid+spatial-count fused in one
     activation with accum_out.
All matmul operands bf16, accumulation fp32.
"""

import numpy as np
from contextlib import ExitStack

import concourse.bass as bass
import concourse.bacc as bacc
import concourse.mybir as mybir
from concourse import tile
from concourse.bass_utils import run_bass_kernel_spmd

BF16 = mybir.dt.bfloat16
F32 = mybir.dt.float32
NP_BF16 = mybir.dt.np(BF16)

B, CIN, H, W = 16, 128, 32, 32
MID, OUT, E = 512, 111, 4
NCORES = 8
SPC = B // NCORES            # samples per core
PAD = 3
HP = H + 2 * PAD             # 38 (padded for the 7x7 branch; smaller kernels share it)
NPIX = H * W                 # 1024
KLIST = (3, 5, 7)
OFFSETS = [(k, dy, dx) for k in KLIST for dy in range(k) for dx in range(k)]
NOFF = len(OFFSETS)          # 83
OH = MID // 2                # output-channel half per pass (256)
GRP = 8                      # offsets per synthesis batch

ADD = mybir.AluOpType.add
MUL = mybir.AluOpType.mult
AF = mybir.ActivationFunctionType
AX = mybir.AxisListType


def _emit(ctx, tc, io):
    nc = tc.nc
    wpool = ctx.enter_context(tc.tile_pool(name="wpool", bufs=3))
    cpool = ctx.enter_context(tc.tile_pool(name="cpool", bufs=4))
    konst = ctx.enter_context(tc.tile_pool(name="konst", bufs=1))
    ypool = ctx.enter_context(tc.tile_pool(name="ypool", bufs=8))
    dpool = ctx.enter_context(tc.tile_pool(name="dpool", bufs=2))
    psp = ctx.enter_context(tc.tile_pool(name="psp", bufs=4, space="PSUM"))

    def load_const(name, shape, dtype, src):
        t = konst.tile(shape, dtype, name=name, tag=name)
        if isinstance(src, bass.DRamTensorHandle):
            src = src[:]
        nc.sync.dma_start(t, src)
        return t

    xs, mk = [], []
    for s in range(SPC):
        xs.append(load_const(f"xs{s}", [CIN, HP * HP], BF16, io["xp"][s]))
        mk.append(load_const(f"mk{s}", [CIN, NPIX], BF16, io["msk"][s]))

    router_sb = load_const("router_sb", [CIN, E], F32, io["router_lhsT"])
    rb_sb = load_const("rb_sb", [SPC, E], F32, io["router_b_bc"])
    fc1_sb = load_const("fc1_sb", [CIN, 4 * 32], F32, io["fc1_lhsT"])
    fc1b_sb = load_const("fc1b_sb", [32, 1], F32, io["fc1_b"])
    fc2_sb = load_const("fc2_sb", [32, MID], F32, io["fc2_lhsT"])
    fc2b_sb = load_const("fc2b_sb", [CIN, 4], F32, io["fc2_b4"])
    pred_sb = load_const("pred_sb", [CIN, 4 * OUT], BF16, io["pred_lhsT"])
    mp_sb = load_const("mp_sb", [CIN, 1], BF16, io["mp_lhsT"])
    mpb_sb = load_const("mpb_sb", [1, 1], F32, io["mp_b"])
    ones32_sb = load_const("ones32_sb", [1, CIN], F32, io["ones_f32"])
    ones16_sb = load_const("ones16_sb", [1, CIN], BF16, io["ones_bf16"])

    # ---------------- routing ----------------
    pooled = konst.tile([CIN, SPC], F32, name="pooled", tag="pooled")
    for s in range(SPC):
        # zeros in the padding don't affect the sum; mean = sum / 1024
        nc.vector.reduce_sum(pooled[:, s : s + 1], xs[s], axis=AX.X)
    lg_ps = psp.tile([SPC, E], F32, name="lg_ps", tag="ps")
    nc.tensor.matmul(lg_ps, pooled, router_sb, start=True, stop=True)
    t_sb = konst.tile([SPC, E], F32, name="t_sb", tag="t_sb")
    nc.vector.scalar_tensor_tensor(t_sb, lg_ps, 1.0 / NPIX, rb_sb, MUL, ADD)
    e_sb = konst.tile([SPC, E], F32, name="e_sb", tag="e_sb")
    nc.scalar.activation(e_sb, t_sb, AF.Exp)
    ssum = konst.tile([SPC, 1], F32, name="ssum", tag="ssum")
    nc.vector.reduce_sum(ssum, e_sb, axis=AX.X)
    rinv = konst.tile([SPC, 1], F32, name="rinv", tag="rinv")
    nc.vector.reciprocal(rinv, ssum)
    r_sb = konst.tile([SPC, E], F32, name="r_sb", tag="r_sb")
    nc.vector.tensor_scalar_mul(r_sb, e_sb, rinv)
    # replicate the SPC*E routing scalars across all 128 partitions via a
    # K=1 matmul with a ones row (SBUF cannot broadcast across partitions).
    r_flat = konst.tile([1, SPC * E], F32, name="r_flat", tag="r_flat")
    nc.sync.dma_start(r_flat, r_sb)
    r_ps = psp.tile([CIN, SPC * E], F32, name="r_ps", tag="ps")
    nc.tensor.matmul(r_ps, ones32_sb, r_flat, start=True, stop=True)
    r128 = konst.tile([CIN, SPC * E], F32, name="r128", tag="r128")
    nc.scalar.copy(r128, r_ps)

    # ---- mask projection (independent of the conv -> do it up front) ----
    mfr = []
    for s in range(SPC):
        mf_ps = psp.tile([1, NPIX], F32, name=f"mfps{s}", tag="ps")
        for hh in range(2):
            nc.tensor.matmul(
                mf_ps[:, 512 * hh : 512 * (hh + 1)], mp_sb,
                mk[s][:, 512 * hh : 512 * (hh + 1)], start=True, stop=True)
        mf_sb = konst.tile([1, NPIX], BF16, name=f"mfsb{s}", tag=f"mfsb{s}")
        nc.scalar.activation(mf_sb, mf_ps, AF.Identity, bias=mpb_sb)
        mfr_ps = psp.tile([CIN, NPIX], F32, name=f"mfrps{s}", tag="ps")
        for hh in range(2):
            nc.tensor.matmul(
                mfr_ps[:, 512 * hh : 512 * (hh + 1)], ones16_sb,
                mf_sb[:, 512 * hh : 512 * (hh + 1)], start=True, stop=True)
        mft = konst.tile([CIN, NPIX], BF16, name=f"mfr{s}", tag=f"mfr{s}")
        nc.scalar.copy(mft, mfr_ps)
        mfr.append(mft)

    # ---------------- dynamic conv: two output-channel passes ----------------
    ca_raw = [
        konst.tile([CIN, 4], F32, name=f"ca{s}", tag=f"ca{s}") for s in range(SPC)
    ]
    y_bf = [[None] * 4 for _ in range(SPC)]
    xv = [xs[s].rearrange("p (h w) -> p h w", h=HP) for s in range(SPC)]

    # synthesis is batched over groups of GRP offsets: one W DMA and 3 fused
    # scalar_tensor_tensor ops per (group, sample) cover GRP offsets at once
    GROUPS = [list(range(a, min(a + GRP, NOFF))) for a in range(0, NOFF, GRP)]
    for p in range(2):
        acc = [
            [
                psp.tile([CIN, NPIX], F32, name=f"acc{p}_{s}_{j}", tag="ps")
                for j in range(2)
            ]
            for s in range(SPC)
        ]
        for gi, grp in enumerate(GROUPS):
            g = len(grp)
            wt = wpool.tile([CIN, GRP * E * OH], BF16, name=f"wt{p}_{gi}", tag="wt")
            nc.sync.dma_start(
                wt[:, : g * E * OH],
                io["wbank"][p][:, grp[0] * E * OH : (grp[0] + g) * E * OH])
            src = wt[:, : g * E * OH].rearrange("p (g e o) -> p g e o", g=g, e=E)
            cws = []
            for s in range(SPC):
                cw = cpool.tile([CIN, GRP * OH], BF16, name=f"cw{p}_{gi}_{s}",
                                tag="cw")
                cwv = cw[:, : g * OH].rearrange("p (g o) -> p g o", g=g)
                for e in range(1, E):
                    nc.vector.scalar_tensor_tensor(
                        cwv, src[:, :, e], r128[:, E * s + e : E * s + e + 1],
                        src[:, :, 0] if e == 1 else cwv, MUL, ADD)
                cws.append(cw)
            for gli, di in enumerate(grp):
                kk, dy, dx = OFFSETS[di]
                first, last = di == 0, di == NOFF - 1
                r0 = PAD - kk // 2 + dy
                c0 = PAD - kk // 2 + dx
                for s in range(SPC):
                    for j in range(2):
                        lhsT = cws[s][:, gli * OH + 128 * j : gli * OH + 128 * (j + 1)]
                        for hh in range(2):
                            rhs = xv[s][:, r0 + 16 * hh : r0 + 16 * hh + 16,
                                        c0 : c0 + 32]
                            nc.tensor.matmul(
                                acc[s][j][:, 512 * hh : 512 * (hh + 1)],
                                lhsT, rhs, start=first, stop=last)
        # drain this pass: PSUM -> bf16 SBUF, channel sums via accum_out
        for s in range(SPC):
            for j in range(2):
                oc = 2 * p + j
                yt = ypool.tile([CIN, NPIX], BF16, name=f"ybf{s}_{oc}", tag="ybf")
                nc.scalar.activation(
                    yt, acc[s][j], AF.Copy, accum_out=ca_raw[s][:, oc : oc + 1])
                y_bf[s][oc] = yt

    # ---------------- per-sample epilogue ----------------
    for s in range(SPC):
        # fc1 (mean folded into host-scaled lhsT) + relu
        fc1_ps = psp.tile([32, 1], F32, name=f"fc1ps{s}", tag="ps")
        for oc in range(4):
            nc.tensor.matmul(
                fc1_ps, fc1_sb[:, 32 * oc : 32 * (oc + 1)],
                ca_raw[s][:, oc : oc + 1], start=(oc == 0), stop=(oc == 3))
        h1 = konst.tile([32, 1], F32, name=f"h1{s}", tag=f"h1{s}")
        nc.scalar.activation(h1, fc1_ps, AF.Relu, bias=fc1b_sb)
        # fc2 + sigmoid
        fc2_ps = psp.tile([CIN, 4], F32, name=f"fc2ps{s}", tag="ps")
        for oc in range(4):
            nc.tensor.matmul(
                fc2_ps[:, oc : oc + 1], fc2_sb[:, 128 * oc : 128 * (oc + 1)],
                h1, start=True, stop=True)
        gat = konst.tile([CIN, 4], F32, name=f"gat{s}", tag=f"gat{s}")
        nc.vector.tensor_add(gat, fc2_ps, fc2b_sb)
        casig = konst.tile([CIN, 4], F32, name=f"casig{s}", tag=f"casig{s}")
        nc.scalar.activation(casig, gat, AF.Sigmoid)
        # fold channel attention into pred lhsT (scale contraction rows)
        spred = konst.tile([CIN, 4 * OUT], BF16, name=f"spred{s}", tag=f"spred{s}")
        for oc in range(4):
            nc.vector.tensor_scalar_mul(
                spred[:, OUT * oc : OUT * (oc + 1)],
                pred_sb[:, OUT * oc : OUT * (oc + 1)], casig[:, oc : oc + 1])
        # pred 1x1 conv: [111, 1024] = sum_oc predT[oc].T @ y[oc]
        pred_ps = psp.tile([OUT, NPIX], F32, name=f"predps{s}", tag="ps")
        for oc in range(4):
            for hh in range(2):
                nc.tensor.matmul(
                    pred_ps[:, 512 * hh : 512 * (hh + 1)],
                    spred[:, OUT * oc : OUT * (oc + 1)],
                    y_bf[s][oc][:, 512 * hh : 512 * (hh + 1)],
                    start=(oc == 0), stop=(oc == 3))
        # d = sigmoid(mf * pred); count = sum_pix d  (fused via accum_out)
        dt_sb = dpool.tile([OUT, NPIX], F32, name=f"dt{s}", tag="dt")
        nc.vector.tensor_mul(dt_sb, pred_ps, mfr[s][0:OUT, :])
        d_sb = dpool.tile([OUT, NPIX], F32, name=f"dsb{s}", tag="dsb")
        cnt = konst.tile([OUT, 1], F32, name=f"cnt{s}", tag=f"cnt{s}")
        nc.scalar.activation(d_sb, dt_sb, AF.Sigmoid, accum_out=cnt)
        nc.sync.dma_start(io["d_out"][s], d_sb)
        nc.sync.dma_start(io["cnt_out"][s], cnt)


def _build_program():
    nc = bacc.Bacc("TRN2", target_bir_lowering=False, debug=False,
                   num_devices=NCORES)
    dp = nc.declare_dram_parameter
    io = {
        "xp": dp("xp", [SPC, CIN, HP * HP], BF16, isOutput=False),
        "msk": dp("msk", [SPC, CIN, NPIX], BF16, isOutput=False),
        "wbank": dp("wbank", [2, CIN, NOFF * E * OH], BF16, isOutput=False),
        "router_lhsT": dp("router_lhsT", [CIN, E], F32, isOutput=False),
        "router_b_bc": dp("router_b_bc", [SPC, E], F32, isOutput=False),
        "fc1_lhsT": dp("fc1_lhsT", [CIN, 4 * 32], F32, isOutput=False),
        "fc1_b": dp("fc1_b", [32, 1], F32, isOutput=False),
        "fc2_lhsT": dp("fc2_lhsT", [32, MID], F32, isOutput=False),
        "fc2_b4": dp("fc2_b4", [CIN, 4], F32, isOutput=False),
        "pred_lhsT": dp("pred_lhsT", [CIN, 4 * OUT], BF16, isOutput=False),
        "mp_lhsT": dp("mp_lhsT", [CIN, 1], BF16, isOutput=False),
        "mp_b": dp("mp_b", [1, 1], F32, isOutput=False),
        "ones_f32": dp("ones_f32", [1, CIN], F32, isOutput=False),
        "ones_bf16": dp("ones_bf16", [1, CIN], BF16, isOutput=False),
        "d_out": dp("d_out", [SPC, OUT, NPIX], F32, isOutput=True),
        "cnt_out": dp("cnt_out", [SPC, OUT, 1], F32, isOutput=True),
    }
    with tile.TileContext(nc) as tc, ExitStack() as ctx:
        _emit(ctx, tc, io)
    nc.compile()
    return nc


_PROGRAM = None


def _get_program():
    global _PROGRAM
    if _PROGRAM is None:
        _PROGRAM = _build_program()
    return _PROGRAM


def prep_inputs(x, mask, w3, w5, w7, router_w, router_b, fc1_w, fc1_b,
                fc2_w, fc2_b, maskproj_w, maskproj_b, pred_w):
    """Host-side layout prep (padding / transposes / dtype casts only)."""
    f32 = np.float32
    x = np.asarray(x, f32)
    xp = np.zeros((B, CIN, HP, HP), f32)
    xp[:, :, PAD : PAD + H, PAD : PAD + W] = x
    xp = np.ascontiguousarray(xp.reshape(B, CIN, HP * HP)).astype(NP_BF16)
    mk = np.ascontiguousarray(
        np.asarray(mask, f32).reshape(B, CIN, NPIX)).astype(NP_BF16)

    # layout: wbank[p][i, di*E*OH + e*OH + o'] = Wdelta[e, p*OH+o', i, dy, dx]
    wbank = np.empty((2, CIN, NOFF * E * OH), NP_BF16)
    di = 0
    for wk, k in ((w3, 3), (w5, 5), (w7, 7)):
        wk = np.asarray(wk, f32)            # [E, MID, CIN, k, k]
        dk = wk.copy()
        dk[1:] -= wk[:1]                    # base + deltas (softmax sums to 1)
        dkt = np.transpose(dk, (3, 4, 2, 0, 1))  # [k, k, CIN, E, MID]
        for dy in range(k):
            for dx in range(k):
                blk = dkt[dy, dx]           # [CIN, E, MID]
                for p in range(2):
                    wbank[p, :, di * E * OH : (di + 1) * E * OH] = blk[
                        :, :, p * OH : (p + 1) * OH].reshape(CIN, E * OH)
                di += 1
    assert di == NOFF

    shared = {
        "wbank": wbank,
        "router_lhsT": np.ascontiguousarray(np.asarray(router_w, f32).T),
        "router_b_bc": np.ascontiguousarray(
            np.broadcast_to(np.asarray(router_b, f32), (SPC, E))),
        "fc1_lhsT": np.ascontiguousarray(
            (np.asarray(fc1_w, f32).T / NPIX).reshape(4, 128, 32)
            .transpose(1, 0, 2).reshape(128, 4 * 32)),
        "fc1_b": np.asarray(fc1_b, f32).reshape(32, 1),
        "fc2_lhsT": np.ascontiguousarray(np.asarray(fc2_w, f32).T),
        "fc2_b4": np.ascontiguousarray(np.asarray(fc2_b, f32).reshape(4, 128).T),
        "pred_lhsT": np.ascontiguousarray(
            np.asarray(pred_w, f32).T.reshape(4, 128, OUT)
            .transpose(1, 0, 2).reshape(128, 4 * OUT)).astype(NP_BF16),
        "mp_lhsT": np.ascontiguousarray(
            np.asarray(maskproj_w, f32).reshape(1, CIN).T).astype(NP_BF16),
        "mp_b": np.asarray(maskproj_b, f32).reshape(1, 1),
        "ones_f32": np.ones((1, CIN), f32),
        "ones_bf16": np.ones((1, CIN), NP_BF16),
    }
    in_maps = []
    for c in range(NCORES):
        sl = slice(c * SPC, (c + 1) * SPC)
        m = dict(shared)
        m["xp"] = np.ascontiguousarray(xp[sl])
        m["msk"] = np.ascontiguousarray(mk[sl])
        in_maps.append(m)
    return in_maps


def assemble_outputs(per_core):
    d = np.concatenate([np.asarray(r["d_out"]) for r in per_core], axis=0)
    cnt = np.concatenate([np.asarray(r["cnt_out"]) for r in per_core], axis=0)
    d = d.reshape(B, OUT, H, W).astype(np.float32)
    cnt = cnt.reshape(B, OUT).astype(np.float32)
    return cnt, d


def kernel(**inputs):
    nc = _get_program()
    in_maps = prep_inputs(**inputs)
    res = run_bass_kernel_spmd(nc, in_maps, list(range(NCORES)))
    return assemble_outputs(res.results)
